# revision 32
# baseline (speedup 1.0000x reference)
"""Trainium2 Bass kernel for NoTPAttention (dense transformer block:
fused QKV projection -> multi-head attention -> output projection).

Sharding (8 NeuronCores): core c handles batch b = c // 4 and the 4 heads
g = 4*(c % 4) .. 4*(c % 4)+3 (head-parallel tensor parallelism).  Each core
computes its heads' partial out-projection [S, H] in bf16; the host sums the
4 partials per batch in fp32 and adds the (folded) biases.

Numerics (validated against a bit-accurate host simulation of every
quantization point; absmax-relative error 1.84e-2 vs the 2e-2 gate):
  * q/k projections run in fp8(e4m3) with DoubleRow perf mode -- the PE
    packs two 128-deep contraction tiles per pass (~1.6x the bf16 rate).
    Weights are pre-scaled x16 on the host so e4m3 never goes subnormal;
    the PSUM drain descales.  Score noise from this is smoothed by softmax.
  * The softmax-denominator ones-matmul also runs fp8 DoubleRow, on an
    fp8 copy of exp(s) produced by a DVE cast.  Denominators are sums of
    2048 positive terms, so the fp8 noise attenuates by ~sqrt(2048).
  * Everything downstream of the softmax averaging is noise-critical and
    stays bf16: the v projection (x in bf16), the PV matmul (exp probs and
    v in bf16), and the out-projection.  fp8 there measures 2.5-3.9e-2:
    over the gate.  DoubleRow cannot apply to the scores matmul at all
    (its contraction is a single 128-deep tile).

Softmax is computed without max-subtraction (scores are bounded, |s| < ~3.5)
with the normalization deferred to the attention *output*:
    attnT[d, q] = (sum_k v[k, d] * exp(sT[k, q])) / (sum_k exp(sT[k, q]))
The denominator's ones-matmul lands it already broadcast across partitions.
The v-bias is dropped in-kernel: after normalization it contributes exactly
b_v to every row, so the host folds w_out @ b_v into the output bias.

Layout notes: qT/kT/attnT live as [128 (head-dim), head, seq] so every
matmul contracts over a full 128-partition tile with no transposes anywhere.
All dram inputs are pre-tiled on the host into partition-major order so
every input DMA moves 2-16KB contiguous runs per partition (the naive
(ht p)->p rearrange gives 512B runs and ~3x slower DMA, which starved the
PE at startup and between x chunks).  The q/k fp8 weights and bf16 wv share
SBUF slots with the attention exp-buffers (tag "e"); the fp8 exp copies
share slots with the streamed fp8 x chunks (tag "x8").

Scheduling notes (each measured on HW):
  * dummy ones-matmuls run during the startup DMA window so the PE's HAM
    clock gate is already at 2.4 GHz when the first real matmul issues;
  * the per-qc-group out-projection is emitted one pipeline slot late so
    the PE never stalls on the z -> reciprocal -> normalize chain;
  * the e->fp8 DVE casts are emitted after the previous chunk's normalize
    (DVE order: recip, mul, casts) -- any other order puts them on the
    projection's or the denominator-matmul's critical path;
  * the final (3,3) attention chunk runs as two 256-query halves with the
    last out-projection interleaved, halving the end-of-kernel drain;
  * out-projection drains collect 4 PSUM banks into one [P, 4, 512] tile
    and issue a single 256KB-contiguous DMA per 128-row block;
  * phase-1 qk groups lead v groups by two x-chunks, so the 4MB bf16
    x/wv transfers that feed the v matmuls get ~17us of DMA slack instead
    of a zero-margin race (this was the only run-to-run timing jitter --
    the attention phase is cycle-deterministic at 194.28us).
"""

import numpy as np
import ml_dtypes

B, S, H = 2, 2048, 2048
NH, HD = 16, 128
P = 128
HT = H // P            # 16 hidden-dim tiles
G = 4                  # heads per core
GH = G * HD            # 512: head-group width per core
SCALE = 1.0 / float(np.sqrt(HD))
N_CORES = 8
XC = 512               # phase-1 x streaming chunk (s elements)
QC = 512               # attention query chunk
KT = S // P            # 16 key tiles
WS = 16.0              # fp8 weight pre-scale (avoids e4m3 subnormals)

_CACHE = {}


def _build():
    import concourse.mybir as mybir
    import concourse.tile as tile
    from concourse import bacc

    dt = mybir.dt
    Alu = mybir.AluOpType
    Act = mybir.ActivationFunctionType
    DR = mybir.MatmulPerfMode.DoubleRow

    nc = bacc.Bacc("TRN2", target_bir_lowering=False, debug=False,
                   enable_asserts=False)

    NXC = S // XC      # 4
    NQC = S // QC      # 4

    # all inputs pre-tiled host-side to partition-major contiguous layouts
    x8_d = nc.dram_tensor("x8", [P, NXC, HT, XC], dt.float8e4,
                          kind="ExternalInput").ap()
    xb_d = nc.dram_tensor("xb", [P, NXC, HT, XC], dt.bfloat16,
                          kind="ExternalInput").ap()
    wqt_d = nc.dram_tensor("wqt", [P, G, HT, HD], dt.float8e4,
                           kind="ExternalInput").ap()
    wkt_d = nc.dram_tensor("wkt", [P, G, HT, HD], dt.float8e4,
                           kind="ExternalInput").ap()
    wvt_d = nc.dram_tensor("wvt", [P, HT, GH], dt.bfloat16,
                           kind="ExternalInput").ap()
    bqs_d = nc.dram_tensor("bqs", [P, G], dt.float32, kind="ExternalInput").ap()
    bk_d = nc.dram_tensor("bk", [P, G], dt.float32, kind="ExternalInput").ap()
    wot_d = nc.dram_tensor("wot", [P, G, H], dt.bfloat16,
                           kind="ExternalInput").ap()
    out_d = nc.dram_tensor("partial", [S // P, P, H], dt.bfloat16,
                           kind="ExternalOutput").ap()

    with tile.TileContext(nc) as tc:
        with (
            tc.tile_pool(name="consts", bufs=1) as consts,
            tc.tile_pool(name="wpool", bufs=1) as wpool,
            tc.tile_pool(name="xpool", bufs=2) as xpool,
            tc.tile_pool(name="big", bufs=1) as big,
            tc.tile_pool(name="epool", bufs=3) as epool,
            tc.tile_pool(name="small", bufs=2) as small,
            tc.tile_pool(name="psum", bufs=2, space="PSUM") as psum,
        ):
            # --- startup DMAs, critical-path (head-0 q weights + first x8
            # chunk) first; every transfer is contiguous per partition.  The
            # first transfers are triggered from the scalar engine's HWDGE
            # queue, which comes out of runtime init several microseconds
            # before the sync engine's. ---
            wq_sb = epool.tile([P, G, HT, HD], dt.float8e4, tag="e",
                               name="wq_sb")
            nc.scalar.dma_start(wq_sb[:, 0], wqt_d[:, 0])
            x8s0 = xpool.tile([P, HT, XC], dt.float8e4, tag="x8",
                              name="x8s0", bufs=3)
            for q4 in range(4):
                nc.scalar.dma_start(x8s0[:, 4 * q4:4 * (q4 + 1), :],
                                    x8_d[:, 0, 4 * q4:4 * (q4 + 1), :])
            nc.sync.dma_start(wq_sb[:, 1:], wqt_d[:, 1:])
            bqs_sb = consts.tile([P, G], dt.float32)
            nc.sync.dma_start(bqs_sb[:], bqs_d)
            bk_sb = consts.tile([P, G], dt.float32)
            nc.sync.dma_start(bk_sb[:], bk_d)
            ones_sb = consts.tile([P, 2, P], dt.float8e4)
            nc.vector.memset(ones_sb[:], 1.0)
            # HAM warmup: the PE clock-gate defaults to 1.2 GHz until it sees
            # ~3.4us of sustained matmul activity.  Dummy ones-matmuls during
            # the otherwise-idle startup DMA window un-throttle it so the
            # real matmuls run at 2.4 GHz from the first instruction.
            warm_ps = psum.tile([P, 2 * P], dt.float32, tag="mm")
            for _ in range(24):
                nc.tensor.matmul(warm_ps, ones_sb[:, 0, :], ones_sb[:, :, :],
                                 start=True, stop=True)
            wk_sb = epool.tile([P, G, HT, HD], dt.float8e4, tag="e",
                               name="wk_sb")
            nc.sync.dma_start(wk_sb[:], wkt_d)
            x8s1 = xpool.tile([P, HT, XC], dt.float8e4, tag="x8",
                              name="x8s1", bufs=3)
            nc.sync.dma_start(x8s1[:], x8_d[:, 1])
            xbs0 = xpool.tile([P, HT, XC], dt.bfloat16, tag="xb",
                              name="xbs0")
            nc.sync.dma_start(xbs0[:], xb_d[:, 0])
            wv_sb = epool.tile([P, HT, GH], dt.bfloat16, tag="e", name="wv_sb")
            nc.sync.dma_start(wv_sb[:], wvt_d)

            qt_sb = big.tile([P, G, S], dt.bfloat16)   # q^T, scale+bias applied
            kt_sb = big.tile([P, G, S], dt.bfloat16)   # k^T, bias applied
            v_sb = big.tile([P, KT, GH], dt.bfloat16)  # v natural [s, o]
            at_sb = big.tile([P, G, S], dt.bfloat16)   # attn output^T

            # ------- Phase 1: QKV projections (q/k fp8 DoubleRow, v bf16) ---
            # qk groups lead v groups by ~2 x-chunks so the early PE stream
            # only ever waits on the small fp8 tensors while the big bf16
            # x/wv transfers stream in behind them.
            def qk_group(xc, x8s):
                sl = slice(xc * XC, (xc + 1) * XC)
                for h in range(G):
                    psq = psum.tile([P, 512], dt.float32, tag="mm")
                    for t2 in range(HT // 2):
                        nc.tensor.matmul(psq,
                                         wq_sb[:, h, 2 * t2:2 * t2 + 2, :],
                                         x8s[:, 2 * t2:2 * t2 + 2, :],
                                         start=(t2 == 0),
                                         stop=(t2 == HT // 2 - 1),
                                         perf_mode=DR)
                    nc.vector.tensor_scalar(qt_sb[:, h, sl], psq,
                                            SCALE / WS, bqs_sb[:, h:h + 1],
                                            Alu.mult, Alu.add)
                for h in range(G):
                    psk = psum.tile([P, 512], dt.float32, tag="mm")
                    for t2 in range(HT // 2):
                        nc.tensor.matmul(psk,
                                         wk_sb[:, h, 2 * t2:2 * t2 + 2, :],
                                         x8s[:, 2 * t2:2 * t2 + 2, :],
                                         start=(t2 == 0),
                                         stop=(t2 == HT // 2 - 1),
                                         perf_mode=DR)
                    nc.vector.tensor_scalar(kt_sb[:, h, sl], psk,
                                            1.0 / WS, bk_sb[:, h:h + 1],
                                            Alu.mult, Alu.add)

            def v_group(xc, xbs):
                for sv in range(XC // P):
                    sm = xc * (XC // P) + sv
                    psv = psum.tile([P, 512], dt.float32, tag="mm")
                    for ht in range(HT):
                        nc.tensor.matmul(psv,
                                         xbs[:, ht, sv * P:(sv + 1) * P],
                                         wv_sb[:, ht, :],
                                         start=(ht == 0), stop=(ht == HT - 1))
                    nc.vector.tensor_copy(out=v_sb[:, sm, :], in_=psv)

            # qk groups lead v groups by ~2 x-chunks: the v work (which needs
            # the big bf16 x/wv transfers) starts ~35us into the PE stream,
            # giving those DMAs slack instead of the ~0-margin race that made
            # phase-1 timing jitter run to run.
            x8t = {0: x8s0, 1: x8s1}
            xbt = {0: xbs0}
            for kind, xc in [("qk", 0), ("qk", 1), ("v", 0), ("qk", 2),
                             ("v", 1), ("qk", 3), ("v", 2), ("v", 3)]:
                if kind == "qk":
                    if xc not in x8t:
                        x8s = xpool.tile([P, HT, XC], dt.float8e4, tag="x8",
                                         name="x8s", bufs=3)
                        nc.sync.dma_start(x8s[:], x8_d[:, xc])
                        x8t[xc] = x8s
                    qk_group(xc, x8t[xc])
                else:
                    if xc not in xbt:
                        xbs = xpool.tile([P, HT, XC], dt.bfloat16, tag="xb",
                                         name="xbs")
                        nc.sync.dma_start(xbs[:], xb_d[:, xc])
                        xbt[xc] = xbs
                    v_group(xc, xbt[xc])

            # out-proj weights: needed only from the first proj (~mid-kernel)
            wo_sb = wpool.tile([P, G, H], dt.bfloat16)
            nc.sync.dma_start(wo_sb[:], wot_d)

            # -------- Phase 2+3: attention + out-proj (sw-pipelined) --------
            def emit_st_exp(h, q0, qw):
                # ST^T = k^T.T @ q^T per 128-key tile; exp on ACT in 2-bank
                # batches (halves the 352-cycle per-ACTIVATE overhead).
                e_sb = epool.tile([P, KT, qw], dt.bfloat16, tag="e",
                                  name="e_sb")
                for km in range(0, KT, 2):
                    ps = psum.tile([P, 2, qw], dt.float32, tag="st")
                    for j in range(2):
                        nc.tensor.matmul(ps[:, j, :],
                                         kt_sb[:, h, (km + j) * P:(km + j + 1) * P],
                                         qt_sb[:, h, q0:q0 + qw],
                                         start=True, stop=True)
                    nc.scalar.activation(e_sb[:, km:km + 2, :], ps, Act.Exp)
                return e_sb

            def emit_e8_cast(e_sb, qw):
                # fp8 copy of exp(s) for the DoubleRow denominator matmul.
                # Emitted *after* the previous chunk's normalize so the DVE
                # casts never sit ahead of it in the queue and stall the
                # projection matmuls.
                e8_sb = xpool.tile([P, KT, qw], dt.float8e4, tag="x8",
                                   name="e8_sb", bufs=3)
                for km in range(0, KT, 2):
                    nc.vector.tensor_copy(out=e8_sb[:, km:km + 2, :],
                                          in_=e_sb[:, km:km + 2, :])
                return e8_sb

            def emit_pv_z_norm(h, q0, qw, e_sb, e8_sb):
                pv = psum.tile([P, qw], dt.float32, tag="pv", bufs=1)
                for km in range(KT):
                    nc.tensor.matmul(pv, v_sb[:, km, h * HD:(h + 1) * HD],
                                     e_sb[:, km, :],
                                     start=(km == 0), stop=(km == KT - 1))
                # softmax denominator: fp8 DoubleRow ones-matmul sums over
                # keys (partitions) and lands it broadcast across partitions
                z = psum.tile([P, qw], dt.float32, tag="z", bufs=1)
                for k2 in range(KT // 2):
                    nc.tensor.matmul(z, ones_sb[:],
                                     e8_sb[:, 2 * k2:2 * k2 + 2, :],
                                     start=(k2 == 0), stop=(k2 == KT // 2 - 1),
                                     perf_mode=DR)
                zi = small.tile([P, qw], dt.float32, tag="zi")
                nc.vector.reciprocal_approx_fast(out=zi[:], in_=z)
                nc.vector.tensor_mul(out=at_sb[:, h, q0:q0 + qw],
                                     in0=pv, in1=zi[:])

            def emit_proj(sms, last=False):
                for sm in sms:
                    ob = small.tile([P, 4, 512], dt.bfloat16, tag="ob", bufs=2)
                    for oc in range(H // 512):
                        pp = psum.tile([P, 512], dt.float32, tag="mm")
                        for g in range(G):
                            nc.tensor.matmul(pp,
                                             at_sb[:, g, sm * P:(sm + 1) * P],
                                             wo_sb[:, g, oc * 512:(oc + 1) * 512],
                                             start=(g == 0), stop=(g == G - 1))
                        # near the tail, split the drain copies across DVE and
                        # ACT so the final drains aren't serialized on one
                        # engine (Copy is in every ACT table set: no reload)
                        if last and oc % 2 == 1:
                            nc.scalar.copy(ob[:, oc, :], pp)
                        else:
                            nc.vector.tensor_copy(out=ob[:, oc, :], in_=pp)
                        if last:
                            # stream each block out as soon as it drains so
                            # the final DMA isn't serialized behind all four
                            # drain copies
                            nc.sync.dma_start(
                                out_d[sm, :, oc * 512:(oc + 1) * 512],
                                ob[:, oc, :])
                    if not last:
                        nc.sync.dma_start(out_d[sm], ob[:])

            # chunk schedule: the final (3,3) chunk runs as two 256-query
            # halves so the last exp/PV/proj drain a half-chunk deep, and the
            # final projection is emitted per query-half.
            ops = []
            for qc in range(NQC):
                for h in range(G):
                    if (h, qc) == (G - 1, NQC - 1):
                        for q4 in range(4):
                            ops.append((h, qc * QC + q4 * (QC // 4), QC // 4))
                    else:
                        ops.append((h, qc * QC, QC))

            # the projection for a finished qc group is delayed one pipeline
            # slot (emitted after the NEXT chunk's score matmuls) so the PE
            # never waits on the z->reciprocal->normalize chain at a group
            # boundary.
            prev = None
            pending = None
            for op in ops:
                h, q0, qw = op
                e = emit_st_exp(h, q0, qw)
                if pending is not None:
                    emit_proj(pending)
                    pending = None
                if prev is not None:
                    ph, pq0, pqw, pe, pe8 = prev
                    emit_pv_z_norm(ph, pq0, pqw, pe, pe8)
                    if ph == G - 1:
                        pending = range(pq0 // P, (pq0 + pqw) // P)
                e8 = emit_e8_cast(e, qw)
                prev = (h, q0, qw, e, e8)
            ph, pq0, pqw, pe, pe8 = prev
            if pending is not None:
                emit_proj(pending)
            emit_pv_z_norm(ph, pq0, pqw, pe, pe8)
            emit_proj(range(pq0 // P, (pq0 + pqw) // P), last=True)

    nc.compile()
    return nc


def _get_nc():
    if "nc" not in _CACHE:
        _CACHE["nc"] = _build()
    return _CACHE["nc"]


def _make_in_maps(x, w_qkv, b_qkv, w_out):
    bf = ml_dtypes.bfloat16
    f8 = ml_dtypes.float8_e4m3
    f32 = np.float32
    NXC = S // XC
    in_maps = []
    for c in range(N_CORES):
        b = c // 4
        g = c % 4
        lo = GH * g
        hi = GH * (g + 1)
        xt = np.ascontiguousarray(x[b].T)               # [H, S]
        # [H, S] -> [P, NXC, HT, XC] partition-major tiling
        xtile = xt.reshape(HT, P, NXC, XC).transpose(1, 2, 0, 3)
        x8 = np.ascontiguousarray(xtile).astype(f8)
        xb = np.ascontiguousarray(xtile).astype(bf)

        def wtile_qk(w):                                 # [GH, H] rows
            # w.T [H, GH] -> [P, G, HT, HD]
            return np.ascontiguousarray(
                (w.T * WS).reshape(HT, P, G, HD).transpose(1, 2, 0, 3)
            ).astype(f8)

        wqt = wtile_qk(w_qkv[lo:hi, :])
        wkt = wtile_qk(w_qkv[H + lo:H + hi, :])
        wvt = np.ascontiguousarray(
            w_qkv[2 * H + lo:2 * H + hi, :].T.reshape(HT, P, GH)
            .transpose(1, 0, 2)).astype(bf)
        bqs = np.ascontiguousarray(
            (b_qkv[lo:hi] * SCALE).astype(f32).reshape(G, P).T)
        bk = np.ascontiguousarray(
            b_qkv[H + lo:H + hi].astype(f32).reshape(G, P).T)
        wot = np.ascontiguousarray(
            w_out[:, lo:hi].T.reshape(G, P, H).transpose(1, 0, 2)).astype(bf)
        in_maps.append({"x8": x8, "xb": xb, "wqt": wqt, "wkt": wkt,
                        "wvt": wvt, "bqs": bqs, "bk": bk, "wot": wot})
    return in_maps


def kernel(x, w_qkv, b_qkv, w_out, b_out):
    import os
    import sys

    x = np.asarray(x, dtype=np.float32)
    w_qkv = np.asarray(w_qkv, dtype=np.float32)
    b_qkv = np.asarray(b_qkv, dtype=np.float32)
    w_out = np.asarray(w_out, dtype=np.float32)
    b_out = np.asarray(b_out, dtype=np.float32)

    from concourse.bass_utils import run_bass_kernel_spmd

    # NTFF tracing under axon needs the antenv.axon_hooks shim (test.py
    # installs it); without it a stray BASS_TRACE=1 in the environment would
    # crash the run — disable tracing in that case.
    if "antenv.axon_hooks" not in sys.modules:
        os.environ["BASS_NEVER_TRACE"] = "1"

    nc = _get_nc()
    in_maps = _make_in_maps(x, w_qkv, b_qkv, w_out)
    res = run_bass_kernel_spmd(nc, in_maps, core_ids=list(range(N_CORES)))
    _CACHE["last_results"] = res
    partials = [r["partial"] for r in res.results]

    bv = b_qkv[2 * H:3 * H]
    bias = b_out + w_out @ bv          # folded v-bias contribution
    out = np.empty((B, S, H), np.float32)
    for b in range(B):
        acc = partials[4 * b].astype(np.float32)
        for g in range(1, 4):
            acc += partials[4 * b + g].astype(np.float32)
        # un-tile [S/P, P, H] -> [S, H]
        out[b] = acc.reshape(S, H) + bias
    return out


# revision 33
# speedup vs baseline: 1.0074x; 1.0074x over previous
"""Trainium2 Bass kernel for NoTPAttention (dense transformer block:
fused QKV projection -> multi-head attention -> output projection).

Sharding (8 NeuronCores): core c handles batch b = c // 4 and the 4 heads
g = 4*(c % 4) .. 4*(c % 4)+3 (head-parallel tensor parallelism).  Each core
computes its heads' partial out-projection [S, H] in bf16; the host sums the
4 partials per batch in fp32 and adds the (folded) biases.

Numerics (validated against a bit-accurate host simulation of every
quantization point; absmax-relative error 1.84e-2 vs the 2e-2 gate):
  * q/k projections run in fp8(e4m3) with DoubleRow perf mode -- the PE
    packs two 128-deep contraction tiles per pass (~1.6x the bf16 rate).
    Weights are pre-scaled x16 on the host so e4m3 never goes subnormal;
    the PSUM drain descales.  Score noise from this is smoothed by softmax.
  * The softmax-denominator ones-matmul also runs fp8 DoubleRow, on an
    fp8 copy of exp(s) produced by a DVE cast.  Denominators are sums of
    2048 positive terms, so the fp8 noise attenuates by ~sqrt(2048).
  * Everything downstream of the softmax averaging is noise-critical and
    stays bf16: the v projection (x in bf16), the PV matmul (exp probs and
    v in bf16), and the out-projection.  fp8 there measures 2.5-3.9e-2:
    over the gate.  DoubleRow cannot apply to the scores matmul at all
    (its contraction is a single 128-deep tile).

Softmax is computed without max-subtraction (scores are bounded, |s| < ~3.5)
with the normalization deferred to the attention *output*:
    attnT[d, q] = (sum_k v[k, d] * exp(sT[k, q])) / (sum_k exp(sT[k, q]))
The denominator's ones-matmul lands it already broadcast across partitions.
The v-bias is dropped in-kernel: after normalization it contributes exactly
b_v to every row, so the host folds w_out @ b_v into the output bias.

Layout notes: qT/kT/attnT live as [128 (head-dim), head, seq] so every
matmul contracts over a full 128-partition tile with no transposes anywhere.
All dram inputs are pre-tiled on the host into partition-major order so
every input DMA moves 2-16KB contiguous runs per partition (the naive
(ht p)->p rearrange gives 512B runs and ~3x slower DMA, which starved the
PE at startup and between x chunks).  The q/k fp8 weights and bf16 wv share
SBUF slots with the attention exp-buffers (tag "e"); the fp8 exp copies
share slots with the streamed fp8 x chunks (tag "x8").

Scheduling notes (each measured on HW):
  * dummy ones-matmuls run during the startup DMA window so the PE's HAM
    clock gate is already at 2.4 GHz when the first real matmul issues;
  * the per-qc-group out-projection is emitted one pipeline slot late so
    the PE never stalls on the z -> reciprocal -> normalize chain;
  * the e->fp8 DVE casts are emitted after the previous chunk's normalize
    (DVE order: recip, mul, casts) -- any other order puts them on the
    projection's or the denominator-matmul's critical path;
  * the final (3,3) attention chunk runs as two 256-query halves with the
    last out-projection interleaved, halving the end-of-kernel drain;
  * out-projection drains collect 4 PSUM banks into one [P, 4, 512] tile
    and issue a single 256KB-contiguous DMA per 128-row block;
  * phase-1 qk groups lead v groups by two x-chunks, so the 4MB bf16
    x/wv transfers that feed the v matmuls get ~17us of DMA slack instead
    of a zero-margin race (this was the only run-to-run timing jitter --
    the attention phase is cycle-deterministic at 194.28us).
"""

import numpy as np
import ml_dtypes

B, S, H = 2, 2048, 2048
NH, HD = 16, 128
P = 128
HT = H // P            # 16 hidden-dim tiles
G = 4                  # heads per core
GH = G * HD            # 512: head-group width per core
SCALE = 1.0 / float(np.sqrt(HD))
N_CORES = 8
XC = 512               # phase-1 x streaming chunk (s elements)
QC = 512               # attention query chunk
KT = S // P            # 16 key tiles
WS = 16.0              # fp8 weight pre-scale (avoids e4m3 subnormals)

_CACHE = {}


def _build():
    import concourse.mybir as mybir
    import concourse.tile as tile
    from concourse import bacc

    dt = mybir.dt
    Alu = mybir.AluOpType
    Act = mybir.ActivationFunctionType
    DR = mybir.MatmulPerfMode.DoubleRow

    nc = bacc.Bacc("TRN2", target_bir_lowering=False, debug=False,
                   enable_asserts=False)

    NXC = S // XC      # 4
    NQC = S // QC      # 4

    # all inputs pre-tiled host-side to partition-major contiguous layouts
    x8_d = nc.dram_tensor("x8", [P, NXC, HT, XC], dt.float8e4,
                          kind="ExternalInput").ap()
    xb_d = nc.dram_tensor("xb", [P, NXC, HT, XC], dt.bfloat16,
                          kind="ExternalInput").ap()
    wqt_d = nc.dram_tensor("wqt", [P, G, HT, HD], dt.float8e4,
                           kind="ExternalInput").ap()
    wkt_d = nc.dram_tensor("wkt", [P, G, HT, HD], dt.float8e4,
                           kind="ExternalInput").ap()
    wvt_d = nc.dram_tensor("wvt", [P, HT, GH], dt.bfloat16,
                           kind="ExternalInput").ap()
    bqs_d = nc.dram_tensor("bqs", [P, G], dt.float32, kind="ExternalInput").ap()
    bk_d = nc.dram_tensor("bk", [P, G], dt.float32, kind="ExternalInput").ap()
    wot_d = nc.dram_tensor("wot", [P, G, H], dt.bfloat16,
                           kind="ExternalInput").ap()
    out_d = nc.dram_tensor("partial", [S // P, P, H], dt.bfloat16,
                           kind="ExternalOutput").ap()

    with tile.TileContext(nc) as tc:
        with (
            tc.tile_pool(name="consts", bufs=1) as consts,
            tc.tile_pool(name="wpool", bufs=1) as wpool,
            tc.tile_pool(name="xpool", bufs=2) as xpool,
            tc.tile_pool(name="big", bufs=1) as big,
            tc.tile_pool(name="epool", bufs=3) as epool,
            tc.tile_pool(name="small", bufs=2) as small,
            tc.tile_pool(name="psum", bufs=2, space="PSUM") as psum,
        ):
            # --- startup DMAs, critical-path (head-0 q weights + first x8
            # chunk) first; every transfer is contiguous per partition.  The
            # first transfers are triggered from the scalar engine's HWDGE
            # queue, which comes out of runtime init several microseconds
            # before the sync engine's. ---
            wq_sb = epool.tile([P, G, HT, HD], dt.float8e4, tag="e",
                               name="wq_sb")
            nc.scalar.dma_start(wq_sb[:, 0], wqt_d[:, 0])
            x8s0 = xpool.tile([P, HT, XC], dt.float8e4, tag="x8",
                              name="x8s0", bufs=3)
            for q4 in range(4):
                nc.scalar.dma_start(x8s0[:, 4 * q4:4 * (q4 + 1), :],
                                    x8_d[:, 0, 4 * q4:4 * (q4 + 1), :])
            nc.sync.dma_start(wq_sb[:, 1:], wqt_d[:, 1:])
            bqs_sb = consts.tile([P, G], dt.float32)
            nc.sync.dma_start(bqs_sb[:], bqs_d)
            bk_sb = consts.tile([P, G], dt.float32)
            nc.sync.dma_start(bk_sb[:], bk_d)
            ones_sb = consts.tile([P, 2, P], dt.float8e4)
            nc.vector.memset(ones_sb[:], 1.0)
            # HAM warmup: the PE clock-gate defaults to 1.2 GHz until it sees
            # ~3.4us of sustained matmul activity.  Dummy ones-matmuls during
            # the otherwise-idle startup DMA window un-throttle it so the
            # real matmuls run at 2.4 GHz from the first instruction.
            warm_ps = psum.tile([P, 2 * P], dt.float32, tag="mm")
            for _ in range(24):
                nc.tensor.matmul(warm_ps, ones_sb[:, 0, :], ones_sb[:, :, :],
                                 start=True, stop=True)
            wk_sb = epool.tile([P, G, HT, HD], dt.float8e4, tag="e",
                               name="wk_sb")
            nc.sync.dma_start(wk_sb[:], wkt_d)
            x8s1 = xpool.tile([P, HT, XC], dt.float8e4, tag="x8",
                              name="x8s1", bufs=3)
            nc.sync.dma_start(x8s1[:], x8_d[:, 1])
            xbs0 = xpool.tile([P, HT, XC], dt.bfloat16, tag="xb",
                              name="xbs0")
            nc.sync.dma_start(xbs0[:], xb_d[:, 0])
            wv_sb = epool.tile([P, HT, GH], dt.bfloat16, tag="e", name="wv_sb")
            nc.sync.dma_start(wv_sb[:], wvt_d)

            qt_sb = big.tile([P, G, S], dt.bfloat16)   # q^T, scale+bias applied
            kt_sb = big.tile([P, G, S], dt.bfloat16)   # k^T, bias applied
            v_sb = big.tile([P, KT, GH], dt.bfloat16)  # v natural [s, o]
            at_sb = big.tile([P, G, S], dt.bfloat16)   # attn output^T

            # ------- Phase 1: QKV projections (q/k fp8 DoubleRow, v bf16) ---
            # qk groups lead v groups by ~2 x-chunks so the early PE stream
            # only ever waits on the small fp8 tensors while the big bf16
            # x/wv transfers stream in behind them.
            def qk_group(xc, x8s):
                sl = slice(xc * XC, (xc + 1) * XC)
                for h in range(G):
                    psq = psum.tile([P, 512], dt.float32, tag="mm")
                    for t2 in range(HT // 2):
                        nc.tensor.matmul(psq,
                                         wq_sb[:, h, 2 * t2:2 * t2 + 2, :],
                                         x8s[:, 2 * t2:2 * t2 + 2, :],
                                         start=(t2 == 0),
                                         stop=(t2 == HT // 2 - 1),
                                         perf_mode=DR)
                    nc.vector.tensor_scalar(qt_sb[:, h, sl], psq,
                                            SCALE / WS, bqs_sb[:, h:h + 1],
                                            Alu.mult, Alu.add)
                for h in range(G):
                    psk = psum.tile([P, 512], dt.float32, tag="mm")
                    for t2 in range(HT // 2):
                        nc.tensor.matmul(psk,
                                         wk_sb[:, h, 2 * t2:2 * t2 + 2, :],
                                         x8s[:, 2 * t2:2 * t2 + 2, :],
                                         start=(t2 == 0),
                                         stop=(t2 == HT // 2 - 1),
                                         perf_mode=DR)
                    nc.vector.tensor_scalar(kt_sb[:, h, sl], psk,
                                            1.0 / WS, bk_sb[:, h:h + 1],
                                            Alu.mult, Alu.add)

            def v_group(xc, xbs):
                for sv in range(XC // P):
                    sm = xc * (XC // P) + sv
                    psv = psum.tile([P, 512], dt.float32, tag="mm")
                    for ht in range(HT):
                        nc.tensor.matmul(psv,
                                         xbs[:, ht, sv * P:(sv + 1) * P],
                                         wv_sb[:, ht, :],
                                         start=(ht == 0), stop=(ht == HT - 1))
                    nc.vector.tensor_copy(out=v_sb[:, sm, :], in_=psv)

            # qk groups lead v groups by ~2 x-chunks: the v work (which needs
            # the big bf16 x/wv transfers) starts ~35us into the PE stream,
            # giving those DMAs slack instead of the ~0-margin race that made
            # phase-1 timing jitter run to run.
            x8t = {0: x8s0, 1: x8s1}
            xbt = {0: xbs0}
            for kind, xc in [("qk", 0), ("qk", 1), ("v", 0), ("qk", 2),
                             ("v", 1), ("qk", 3), ("v", 2), ("v", 3)]:
                if kind == "qk":
                    if xc not in x8t:
                        x8s = xpool.tile([P, HT, XC], dt.float8e4, tag="x8",
                                         name="x8s", bufs=3)
                        nc.sync.dma_start(x8s[:], x8_d[:, xc])
                        x8t[xc] = x8s
                    qk_group(xc, x8t[xc])
                else:
                    if xc not in xbt:
                        xbs = xpool.tile([P, HT, XC], dt.bfloat16, tag="xb",
                                         name="xbs")
                        nc.sync.dma_start(xbs[:], xb_d[:, xc])
                        xbt[xc] = xbs
                    v_group(xc, xbt[xc])

            # out-proj weights: needed only from the first proj (~mid-kernel)
            wo_sb = wpool.tile([P, G, H], dt.bfloat16)
            nc.sync.dma_start(wo_sb[:], wot_d)

            # -------- Phase 2+3: attention + out-proj (sw-pipelined) --------
            def emit_st_exp(h, q0, qw):
                # ST^T = k^T.T @ q^T per 128-key tile; exp on ACT in 2-bank
                # batches (halves the 352-cycle per-ACTIVATE overhead).
                e_sb = epool.tile([P, KT, qw], dt.bfloat16, tag="e",
                                  name="e_sb")
                for km in range(0, KT, 2):
                    ps = psum.tile([P, 2, qw], dt.float32, tag="st")
                    for j in range(2):
                        nc.tensor.matmul(ps[:, j, :],
                                         kt_sb[:, h, (km + j) * P:(km + j + 1) * P],
                                         qt_sb[:, h, q0:q0 + qw],
                                         start=True, stop=True)
                    nc.scalar.activation(e_sb[:, km:km + 2, :], ps, Act.Exp)
                return e_sb

            def emit_e8_cast(e_sb, qw):
                # fp8 copy of exp(s) for the DoubleRow denominator matmul.
                # Emitted *after* the previous chunk's normalize so the DVE
                # casts never sit ahead of it in the queue and stall the
                # projection matmuls.
                e8_sb = xpool.tile([P, KT, qw], dt.float8e4, tag="x8",
                                   name="e8_sb", bufs=3)
                for km in range(0, KT, 2):
                    nc.vector.tensor_copy(out=e8_sb[:, km:km + 2, :],
                                          in_=e_sb[:, km:km + 2, :])
                return e8_sb

            def emit_pv_z_norm(h, q0, qw, e_sb, e8_sb):
                pv = psum.tile([P, qw], dt.float32, tag="pv", bufs=1)
                for km in range(KT):
                    nc.tensor.matmul(pv, v_sb[:, km, h * HD:(h + 1) * HD],
                                     e_sb[:, km, :],
                                     start=(km == 0), stop=(km == KT - 1))
                # softmax denominator: fp8 DoubleRow ones-matmul sums over
                # keys (partitions) and lands it broadcast across partitions
                z = psum.tile([P, qw], dt.float32, tag="z", bufs=1)
                for k2 in range(KT // 2):
                    nc.tensor.matmul(z, ones_sb[:],
                                     e8_sb[:, 2 * k2:2 * k2 + 2, :],
                                     start=(k2 == 0), stop=(k2 == KT // 2 - 1),
                                     perf_mode=DR)
                zi = small.tile([P, qw], dt.float32, tag="zi")
                nc.vector.reciprocal_approx_fast(out=zi[:], in_=z)
                nc.vector.tensor_mul(out=at_sb[:, h, q0:q0 + qw],
                                     in0=pv, in1=zi[:])

            def emit_proj(sms, last=False):
                for sm in sms:
                    ob = small.tile([P, 4, 512], dt.bfloat16, tag="ob", bufs=2)
                    for oc in range(H // 512):
                        pp = psum.tile([P, 512], dt.float32, tag="mm")
                        for g in range(G):
                            nc.tensor.matmul(pp,
                                             at_sb[:, g, sm * P:(sm + 1) * P],
                                             wo_sb[:, g, oc * 512:(oc + 1) * 512],
                                             start=(g == 0), stop=(g == G - 1))
                        # near the tail, split the drain copies across DVE and
                        # ACT so the final drains aren't serialized on one
                        # engine (Copy is in every ACT table set: no reload)
                        if last and oc % 2 == 1:
                            nc.scalar.copy(ob[:, oc, :], pp)
                        else:
                            nc.vector.tensor_copy(out=ob[:, oc, :], in_=pp)
                        if last:
                            # stream each block out as soon as it drains so
                            # the final DMA isn't serialized behind all four
                            # drain copies
                            nc.sync.dma_start(
                                out_d[sm, :, oc * 512:(oc + 1) * 512],
                                ob[:, oc, :])
                    if not last:
                        nc.sync.dma_start(out_d[sm], ob[:])

            # chunk schedule: the final (3,3) chunk runs as two 256-query
            # halves so the last exp/PV/proj drain a half-chunk deep, and the
            # final projection is emitted per query-half.
            ops = []
            for qc in range(NQC):
                for h in range(G):
                    if (h, qc) == (G - 1, NQC - 1):
                        ops.append((h, qc * QC, QC // 2))
                        ops.append((h, qc * QC + QC // 2, QC // 2))
                    else:
                        ops.append((h, qc * QC, QC))

            # the projection for a finished qc group is delayed one pipeline
            # slot (emitted after the NEXT chunk's score matmuls) so the PE
            # never waits on the z->reciprocal->normalize chain at a group
            # boundary.
            prev = None
            pending = None
            for op in ops:
                h, q0, qw = op
                e = emit_st_exp(h, q0, qw)
                if pending is not None:
                    emit_proj(pending)
                    pending = None
                if prev is not None:
                    ph, pq0, pqw, pe, pe8 = prev
                    emit_pv_z_norm(ph, pq0, pqw, pe, pe8)
                    if ph == G - 1:
                        pending = range(pq0 // P, (pq0 + pqw) // P)
                e8 = emit_e8_cast(e, qw)
                prev = (h, q0, qw, e, e8)
            ph, pq0, pqw, pe, pe8 = prev
            if pending is not None:
                emit_proj(pending)
            emit_pv_z_norm(ph, pq0, pqw, pe, pe8)
            emit_proj(range(pq0 // P, (pq0 + pqw) // P), last=True)

    nc.compile()
    return nc


def _get_nc():
    if "nc" not in _CACHE:
        _CACHE["nc"] = _build()
    return _CACHE["nc"]


def _make_in_maps(x, w_qkv, b_qkv, w_out):
    bf = ml_dtypes.bfloat16
    f8 = ml_dtypes.float8_e4m3
    f32 = np.float32
    NXC = S // XC
    in_maps = []
    for c in range(N_CORES):
        b = c // 4
        g = c % 4
        lo = GH * g
        hi = GH * (g + 1)
        xt = np.ascontiguousarray(x[b].T)               # [H, S]
        # [H, S] -> [P, NXC, HT, XC] partition-major tiling
        xtile = xt.reshape(HT, P, NXC, XC).transpose(1, 2, 0, 3)
        x8 = np.ascontiguousarray(xtile).astype(f8)
        xb = np.ascontiguousarray(xtile).astype(bf)

        def wtile_qk(w):                                 # [GH, H] rows
            # w.T [H, GH] -> [P, G, HT, HD]
            return np.ascontiguousarray(
                (w.T * WS).reshape(HT, P, G, HD).transpose(1, 2, 0, 3)
            ).astype(f8)

        wqt = wtile_qk(w_qkv[lo:hi, :])
        wkt = wtile_qk(w_qkv[H + lo:H + hi, :])
        wvt = np.ascontiguousarray(
            w_qkv[2 * H + lo:2 * H + hi, :].T.reshape(HT, P, GH)
            .transpose(1, 0, 2)).astype(bf)
        bqs = np.ascontiguousarray(
            (b_qkv[lo:hi] * SCALE).astype(f32).reshape(G, P).T)
        bk = np.ascontiguousarray(
            b_qkv[H + lo:H + hi].astype(f32).reshape(G, P).T)
        wot = np.ascontiguousarray(
            w_out[:, lo:hi].T.reshape(G, P, H).transpose(1, 0, 2)).astype(bf)
        in_maps.append({"x8": x8, "xb": xb, "wqt": wqt, "wkt": wkt,
                        "wvt": wvt, "bqs": bqs, "bk": bk, "wot": wot})
    return in_maps


def kernel(x, w_qkv, b_qkv, w_out, b_out):
    import os
    import sys

    x = np.asarray(x, dtype=np.float32)
    w_qkv = np.asarray(w_qkv, dtype=np.float32)
    b_qkv = np.asarray(b_qkv, dtype=np.float32)
    w_out = np.asarray(w_out, dtype=np.float32)
    b_out = np.asarray(b_out, dtype=np.float32)

    from concourse.bass_utils import run_bass_kernel_spmd

    # NTFF tracing under axon needs the antenv.axon_hooks shim (test.py
    # installs it); without it a stray BASS_TRACE=1 in the environment would
    # crash the run — disable tracing in that case.
    if "antenv.axon_hooks" not in sys.modules:
        os.environ["BASS_NEVER_TRACE"] = "1"

    nc = _get_nc()
    in_maps = _make_in_maps(x, w_qkv, b_qkv, w_out)
    res = run_bass_kernel_spmd(nc, in_maps, core_ids=list(range(N_CORES)))
    _CACHE["last_results"] = res
    partials = [r["partial"] for r in res.results]

    bv = b_qkv[2 * H:3 * H]
    bias = b_out + w_out @ bv          # folded v-bias contribution
    out = np.empty((B, S, H), np.float32)
    for b in range(B):
        acc = partials[4 * b].astype(np.float32)
        for g in range(1, 4):
            acc += partials[4 * b + g].astype(np.float32)
        # un-tile [S/P, P, H] -> [S, H]
        out[b] = acc.reshape(S, H) + bias
    return out
